# revision 6
# baseline (speedup 1.0000x reference)
"""Trainium2 Bass kernel for LuluAttention (gated GQA attention + RoPE).

Sharding over 8 NeuronCores: core = b*4 + g where b = batch (2), g = head
group (4). Each core computes 4 Q heads + their shared KV head for one batch
element, plus the matching gate slice, and a partial o_proj output
(contraction over its 512 attn dims). Host sums the 4 partials per batch.

All on-chip tensors are kept in transposed layout ([dim, seq]) so the
attention pipeline needs no on-chip transposes:
  qT/kT [d, s]  -> scoresT[sk, sq] = kT_tile.T @ qT_chunk
  softmax over sk (partition dim): denominator via ones-matmul, broadcast of
  the reciprocal via a K=1 matmul.
  v kept straight [s, d] -> attnT[d, sq] = v_tile.T @ probsT
  agT[d, sq] = attnT * gateT * recip  feeds o_proj directly as lhsT.
RoPE rotate-half needs a cross-partition rotation by 64: done with two DMA
copies, signs folded into the host-precomputed sin table.
"""

import numpy as np
import ml_dtypes
from contextlib import ExitStack

import concourse.bass as bass
import concourse.bacc as bacc
import concourse.tile as tile
from concourse import mybir
from concourse.bass_utils import run_bass_kernel_spmd

BF16 = ml_dtypes.bfloat16

HIDDEN = 2048
B = 2
S_FULL = 2048
P = 128
CH = 512               # seq chunk width
QH = 4                 # q heads per core
DQ = QH * P            # 512 q dims per core
KT = HIDDEN // P       # 16 contraction tiles
SCALE = 1.0 / float(np.sqrt(128.0))
ROPE_THETA = 10000.0


def build_program(S=S_FULL):
    f32 = mybir.dt.float32
    bf16 = mybir.dt.bfloat16
    sig = mybir.ActivationFunctionType.Sigmoid
    expf = mybir.ActivationFunctionType.Exp

    NCH = S // CH
    ST = CH // P           # 4 seq sub-tiles per chunk

    nc = bacc.Bacc("TRN2", debug=False, target_bir_lowering=False)

    xT = nc.declare_dram_parameter("xT", [HIDDEN, S], bf16, False)
    wq = nc.declare_dram_parameter("wq", [HIDDEN, DQ], bf16, False)
    wk = nc.declare_dram_parameter("wk", [HIDDEN, P], bf16, False)
    wv = nc.declare_dram_parameter("wv", [HIDDEN, P], bf16, False)
    wg = nc.declare_dram_parameter("wg", [HIDDEN, DQ], bf16, False)
    wo = nc.declare_dram_parameter("wo", [DQ, HIDDEN], bf16, False)
    bg = nc.declare_dram_parameter("bg", [DQ], f32, False)
    cosT = nc.declare_dram_parameter("cosT", [P, S], f32, False)
    sinT = nc.declare_dram_parameter("sinT", [P, S], f32, False)
    msk = nc.declare_dram_parameter("msk", [ST, P, CH], bf16, False)
    out = nc.declare_dram_parameter("out", [S, HIDDEN], f32, True)

    with tile.TileContext(nc) as tc, ExitStack() as ctx:
        wpool = ctx.enter_context(tc.tile_pool(name="weights", bufs=1))
        xpool = ctx.enter_context(tc.tile_pool(name="xchunks", bufs=2))
        qkv = ctx.enter_context(tc.tile_pool(name="qkv", bufs=1))
        work = ctx.enter_context(tc.tile_pool(name="work", bufs=3))
        agp = ctx.enter_context(tc.tile_pool(name="agp", bufs=2))
        outp = ctx.enter_context(tc.tile_pool(name="outp", bufs=2))
        ps_mm = ctx.enter_context(tc.tile_pool(name="ps_mm", bufs=2, space="PSUM"))
        ps_sc = ctx.enter_context(tc.tile_pool(name="ps_sc", bufs=2, space="PSUM"))
        ps_at = ctx.enter_context(tc.tile_pool(name="ps_at", bufs=2, space="PSUM"))
        ps_sm = ctx.enter_context(tc.tile_pool(name="ps_sm", bufs=1, space="PSUM"))

        # ---- persistent loads ----
        wq_sb = wpool.tile([P, KT, DQ], bf16, tag="wq")
        nc.sync.dma_start(out=wq_sb, in_=wq[:, :].rearrange("(kt p) n -> p kt n", p=P))
        wk_sb = wpool.tile([P, KT, P], bf16, tag="wk")
        nc.sync.dma_start(out=wk_sb, in_=wk[:, :].rearrange("(kt p) n -> p kt n", p=P))
        wv_sb = wpool.tile([P, KT, P], bf16, tag="wv")
        nc.sync.dma_start(out=wv_sb, in_=wv[:, :].rearrange("(kt p) n -> p kt n", p=P))
        wg_sb = wpool.tile([P, KT, DQ], bf16, tag="wg")
        nc.sync.dma_start(out=wg_sb, in_=wg[:, :].rearrange("(kt p) n -> p kt n", p=P))
        wo_sb = wpool.tile([P, QH, HIDDEN], bf16, tag="wo")
        nc.sync.dma_start(out=wo_sb, in_=wo[:, :].rearrange("(dt p) n -> p dt n", p=P))
        bg_sb = wpool.tile([P, QH], f32, tag="bg")
        nc.sync.dma_start(out=bg_sb, in_=bg[:].rearrange("(h p) -> p h", p=P))
        cos_sb = wpool.tile([P, S], f32, tag="cos")
        nc.sync.dma_start(out=cos_sb, in_=cosT[:, :])
        sin_sb = wpool.tile([P, S], f32, tag="sin")
        nc.sync.dma_start(out=sin_sb, in_=sinT[:, :])
        msk_sb = wpool.tile([P, ST, CH], bf16, tag="msk")
        nc.sync.dma_start(out=msk_sb, in_=msk[:, :, :].rearrange("o p n -> p o n"))
        ones_pv = wpool.tile([P, 1], bf16, tag="ones_pv")
        nc.vector.memset(ones_pv, 1.0)
        ones_bc = wpool.tile([1, P], f32, tag="ones_bc")
        nc.vector.memset(ones_bc, 1.0)

        # persistent per-core activations (transposed layouts)
        qro = qkv.tile([P, QH, S], bf16, tag="qro")
        kro = qkv.tile([P, S], bf16, tag="kro")
        v_sb = qkv.tile([P, S // P, P], bf16, tag="v")
        gt = qkv.tile([P, QH, S], bf16, tag="gt")

        for c in range(NCH):
            cs = slice(c * CH, (c + 1) * CH)

            # ---- projections for this seq chunk ----
            xc = xpool.tile([P, KT, CH], bf16, tag="xc")
            nc.sync.dma_start(
                out=xc, in_=xT[:, cs].rearrange("(kt p) n -> p kt n", p=P)
            )

            # q heads + k, with RoPE applied out of PSUM
            for qh in range(QH + 1):
                ps = ps_mm.tile([P, CH], f32, tag="proj")
                for kt in range(KT):
                    lhs = (
                        wq_sb[:, kt, qh * P:(qh + 1) * P]
                        if qh < QH
                        else wk_sb[:, kt, :]
                    )
                    nc.tensor.matmul(
                        ps, lhs, xc[:, kt, :], start=(kt == 0), stop=(kt == KT - 1)
                    )
                qf = work.tile([P, CH], f32, tag="qf")
                nc.scalar.copy(out=qf, in_=ps)
                rot = work.tile([P, CH], f32, tag="rot")
                nc.sync.dma_start(out=rot[0:64, :], in_=qf[64:128, :])
                nc.sync.dma_start(out=rot[64:128, :], in_=qf[0:64, :])
                t1 = work.tile([P, CH], f32, tag="t1")
                nc.vector.tensor_mul(t1, qf, cos_sb[:, cs])
                t2 = work.tile([P, CH], f32, tag="t2")
                nc.vector.tensor_mul(t2, rot, sin_sb[:, cs])
                dst = qro[:, qh, cs] if qh < QH else kro[:, cs]
                nc.vector.tensor_add(dst, t1, t2)

            # gate heads: sigmoid(x @ Wg + bg), transposed layout
            for qh in range(QH):
                ps = ps_mm.tile([P, CH], f32, tag="proj")
                for kt in range(KT):
                    nc.tensor.matmul(
                        ps,
                        wg_sb[:, kt, qh * P:(qh + 1) * P],
                        xc[:, kt, :],
                        start=(kt == 0),
                        stop=(kt == KT - 1),
                    )
                nc.scalar.activation(
                    out=gt[:, qh, cs],
                    in_=ps,
                    func=sig,
                    bias=bg_sb[:, qh:qh + 1],
                    scale=1.0,
                )

            # v in straight layout [s, d]
            for st in range(ST):
                s0 = c * ST + st
                ps = ps_mm.tile([P, P], f32, tag="proj")
                for kt in range(KT):
                    nc.tensor.matmul(
                        ps,
                        xc[:, kt, st * P:(st + 1) * P],
                        wv_sb[:, kt, :],
                        start=(kt == 0),
                        stop=(kt == KT - 1),
                    )
                nc.scalar.copy(out=v_sb[:, s0, :], in_=ps)

            # ---- attention for this sq chunk ----
            ag = agp.tile([P, QH, CH], bf16, tag="ag")
            ntiles = (c + 1) * ST
            for qh in range(QH):
                at = ps_at.tile([P, CH], f32, tag="attn")
                dn = ps_sm.tile([1, CH], f32, tag="denom")
                for t in range(ntiles):
                    sc_ps = ps_sc.tile([P, CH], f32, tag="sc")
                    nc.tensor.matmul(
                        sc_ps,
                        kro[:, t * P:(t + 1) * P],
                        qro[:, qh, cs],
                        start=True,
                        stop=True,
                    )
                    pr = work.tile([P, CH], bf16, tag="probs")
                    nc.scalar.activation(out=pr, in_=sc_ps, func=expf, scale=SCALE)
                    o = t - c * ST
                    if o >= 0:
                        nc.vector.tensor_mul(pr, pr, msk_sb[:, o, :])
                    nc.tensor.matmul(
                        at, v_sb[:, t, :], pr,
                        start=(t == 0), stop=(t == ntiles - 1),
                    )
                    nc.tensor.matmul(
                        dn, ones_pv, pr,
                        start=(t == 0), stop=(t == ntiles - 1),
                    )
                rc = work.tile([1, CH], f32, tag="recip")
                nc.vector.reciprocal(rc, dn)
                bc = ps_sm.tile([P, CH], f32, tag="bcast")
                nc.tensor.matmul(bc, ones_bc, rc, start=True, stop=True)
                t3 = work.tile([P, CH], f32, tag="t3")
                nc.vector.tensor_mul(t3, at, gt[:, qh, cs])
                nc.vector.tensor_mul(ag[:, qh, :], t3, bc)

            # ---- partial o_proj for this chunk ----
            for st in range(ST):
                r0 = c * CH + st * P
                for hp in range(HIDDEN // CH // 2):
                    pss = [
                        ps_mm.tile([P, CH], f32, tag="proj", name=f"ops{hi}")
                        for hi in range(2)
                    ]
                    for dt in range(QH):
                        for hi in range(2):
                            h0 = hp * 2 + hi
                            nc.tensor.matmul(
                                pss[hi],
                                ag[:, dt, st * P:(st + 1) * P],
                                wo_sb[:, dt, h0 * CH:(h0 + 1) * CH],
                                start=(dt == 0),
                                stop=(dt == QH - 1),
                            )
                    for hi in range(2):
                        h0 = hp * 2 + hi
                        ob = outp.tile([P, CH], f32, tag="ob")
                        nc.vector.tensor_copy(out=ob, in_=pss[hi])
                        nc.sync.dma_start(
                            out=out[r0:r0 + P, h0 * CH:(h0 + 1) * CH], in_=ob
                        )

    nc.finalize()
    return nc


_PROGRAMS = {}


def _get_program(S=S_FULL):
    if S not in _PROGRAMS:
        _PROGRAMS[S] = build_program(S)
    return _PROGRAMS[S]


def _host_tables(position_ids_b, S):
    pos = np.asarray(position_ids_b, dtype=np.float32)  # [S]
    inv = 1.0 / (ROPE_THETA ** (np.arange(0, P, 2, dtype=np.float32) / P))  # [64]
    ang = np.concatenate([inv, inv]).astype(np.float32)[:, None] * pos[None, :]
    cosT = np.cos(ang).astype(np.float32)
    sgn = np.where(np.arange(P) < 64, -1.0, 1.0).astype(np.float32)
    sinT = (np.sin(ang) * sgn[:, None]).astype(np.float32)
    return cosT, sinT


def _causal_masks():
    o = np.arange(CH // P)[:, None, None]
    r = np.arange(P)[None, :, None]
    j = np.arange(CH)[None, None, :]
    return ((P * o + r) <= j).astype(BF16)


def make_in_maps(x, position_ids, Wq, Wk, Wv, Wo, Wg, bg, S=S_FULL):
    x = np.asarray(x, dtype=np.float32)
    msk = _causal_masks()
    maps = []
    xT_b = [np.ascontiguousarray(x[b, :S].T).astype(BF16) for b in range(B)]
    tabs = [_host_tables(np.asarray(position_ids)[b, :S], S) for b in range(B)]
    Wq = np.asarray(Wq, np.float32)
    Wk = np.asarray(Wk, np.float32)
    Wv = np.asarray(Wv, np.float32)
    Wo = np.asarray(Wo, np.float32)
    Wg = np.asarray(Wg, np.float32)
    bg = np.asarray(bg, np.float32)
    for core in range(8):
        b, g = core // 4, core % 4
        cosT, sinT = tabs[b]
        maps.append({
            "xT": xT_b[b],
            "wq": np.ascontiguousarray(Wq[:, g * DQ:(g + 1) * DQ]).astype(BF16),
            "wk": np.ascontiguousarray(Wk[:, g * P:(g + 1) * P]).astype(BF16),
            "wv": np.ascontiguousarray(Wv[:, g * P:(g + 1) * P]).astype(BF16),
            "wg": np.ascontiguousarray(Wg[:, g * DQ:(g + 1) * DQ]).astype(BF16),
            "wo": np.ascontiguousarray(Wo[g * DQ:(g + 1) * DQ, :]).astype(BF16),
            "bg": np.ascontiguousarray(bg[g * DQ:(g + 1) * DQ]),
            "cosT": cosT,
            "sinT": sinT,
            "msk": msk,
        })
    return maps


def run(inputs, S=S_FULL, trace=False, **kw):
    nc = _get_program(S)
    maps = make_in_maps(S=S, **inputs)
    res = run_bass_kernel_spmd(nc, maps, core_ids=list(range(8)), trace=trace, **kw)
    out = np.zeros((B, S, HIDDEN), np.float32)
    for core in range(8):
        out[core // 4] += np.asarray(res.results[core]["out"], np.float32)
    return out, res


def kernel(x, position_ids, Wq, Wk, Wv, Wo, Wg, bg):
    out, _ = run(dict(x=x, position_ids=position_ids, Wq=Wq, Wk=Wk, Wv=Wv,
                      Wo=Wo, Wg=Wg, bg=bg))
    return out
